# revision 47
# baseline (speedup 1.0000x reference)
"""Multi-head attention (B=8, N=1024, C=768, H=12) on 8 Trainium2 NeuronCores.

Strategy: pure data parallelism over the batch dimension — each of the 8
cores computes full attention for one batch element; weights are
replicated. No collectives needed.

Per-core dataflow (all matmuls expressed as out = lhsT.T @ rhs on the PE):
  1. x loaded in 8 per-token-chunk casting DMAs (bf16); a dense N=512
     warm-up matmul stream raises the PE p-state (HAM flips ~11us), then
     xT is built with PE identity-transposes pipelined behind the DMAs.
  2. qkT = w_qkv[:, :1536].T @ xT  (q,k feature-major, bf16)
     v    = x @ w_qkv[:, 1536:]    (v token-major, bf16, 96-col head slots
                                    with ones at col 64 -> softmax denom)
  3. per head pair (2 heads share a 128-row qkT chunk -> row-tiled K=64):
       both heads' score matmuls for the same n-range write ONE shared
       PSUM tile ([headA 512 | headB 512]) so the two K=64 row-tiles
       (row_grp h0 / h64) issue adjacently and stream CONCURRENTLY.
       expT = exp(scale * scoresT)  (ScalarE, one [128,1024] ACTIVATE per
                                     psum tile; max-subtraction skipped)
       U^T[d,n] += v_aug[m,d] expT[m,n]  (v stationary, 128-wide for FWL;
                                          row 64 = softmax denominator)
       U^T -> token-major via one DMA-XBAR transpose per head; normalize
       with a single broadcast tensor_tensor multiply (recip per token).
  4. aoT via PE transposes per pair; proj is stage-split:
       stage A (mid-loop): yA[t] = sum_{c=0..3} aoT[c] @ wp[c] + b (bf16)
       stage B (tail):     y[t]  = sum_{c=4..5} aoT[c] @ wp[c] + yA[t]
     so only 32 proj matmuls remain after the last pair's normalize.
All matmul operands bf16; accumulation fp32 in PSUM.
"""

import os
import sys

for _p in ("/opt/trn_rl_repo", "/root/.axon_site/_ro/trn_rl_repo"):
    if os.path.isdir(_p) and _p not in sys.path:
        sys.path.append(_p)

from contextlib import ExitStack

import numpy as np

import concourse.bass as bass
import concourse.tile as tile
from concourse import bacc, mybir
from concourse.bass_utils import run_bass_kernel_spmd
from concourse.masks import make_identity

FP = mybir.dt.float32
BF16 = mybir.dt.bfloat16
N_CORES = 8
T = 1024  # tokens per core (batch element)
C = 768
H = 12
D = 64
SCALE = D ** (-0.5)
TC = T // 128  # 8 token chunks
CCH = C // 128  # 6 channel chunks
NPAIR = H // 2  # 6 head pairs
VS = 96  # v columns per head slot (64 data + ones/pad)
CA = 4  # proj stage-A contraction chunks (stage B does CCH - CA)

Exp = mybir.ActivationFunctionType.Exp
Mult = mybir.AluOpType.mult
Div = mybir.AluOpType.divide

def build(n_cores: int = N_CORES):
    nc = bacc.Bacc(
        "TRN2", target_bir_lowering=False, debug=False, num_devices=n_cores
    )
    wdma = nc.gpsimd.dma_start
    x = nc.declare_dram_parameter("x", [T, C], FP, isOutput=False)
    w_qkv = nc.declare_dram_parameter("w_qkv", [C, 3 * C], FP, isOutput=False)
    w_proj = nc.declare_dram_parameter("w_proj", [C, C], FP, isOutput=False)
    b_proj = nc.declare_dram_parameter("b_proj", [C], FP, isOutput=False)
    out = nc.declare_dram_parameter("out", [T, C], FP, isOutput=True)

    xa, wqa, wpa, outa = x.ap(), w_qkv.ap(), w_proj.ap(), out.ap()
    ba = b_proj.ap()
    b_bcast_src = bass.AP(tensor=ba.tensor, offset=ba.offset, ap=[[0, 128]] + ba.ap)

    with tile.TileContext(nc) as tc, ExitStack() as ctx:
        consts = ctx.enter_context(tc.tile_pool(name="consts", bufs=1))
        xs_pool = ctx.enter_context(tc.tile_pool(name="xstage", bufs=4))
        xT_pool = ctx.enter_context(tc.tile_pool(name="xT", bufs=1))
        wq1_pool = ctx.enter_context(tc.tile_pool(name="wq1", bufs=1))
        wq2_pool = ctx.enter_context(tc.tile_pool(name="wq2", bufs=1))
        wp_pool = ctx.enter_context(tc.tile_pool(name="wp", bufs=1))
        # 7 bufs: qkT[p]/qkT[6+p] are dead after scores(p), so pair p+2's
        # f1 writes can safely reuse their slots.
        qk_pool = ctx.enter_context(tc.tile_pool(name="qk", bufs=7))
        v_pool = ctx.enter_context(tc.tile_pool(name="v65", bufs=TC))
        exp_pool = ctx.enter_context(tc.tile_pool(name="expT", bufs=2))
        uT_pool = ctx.enter_context(tc.tile_pool(name="uT", bufs=2))
        at_pool = ctx.enter_context(tc.tile_pool(name="atmp", bufs=2))
        r_pool = ctx.enter_context(tc.tile_pool(name="r", bufs=2))
        # per-pair ao staging (normalized attention out, token-major);
        # 3 bufs since aotp (the reader) trails its pair by an iteration.
        ao_pool = ctx.enter_context(tc.tile_pool(name="ao", bufs=3))
        aoT_pool = ctx.enter_context(tc.tile_pool(name="aoT", bufs=1))
        ya_pool = ctx.enter_context(tc.tile_pool(name="ya", bufs=TC))
        y_pool = ctx.enter_context(tc.tile_pool(name="y", bufs=6))
        # PSUM: accA 2x1 + accB 2x1 + sc 2x2 = 8 banks
        accA = ctx.enter_context(tc.tile_pool(name="accA", bufs=2, space="PSUM"))
        accB = ctx.enter_context(tc.tile_pool(name="accB", bufs=2, space="PSUM"))
        sc_psum = ctx.enter_context(tc.tile_pool(name="sc", bufs=2, space="PSUM"))

        identity_h = consts.tile([128, 128], BF16)
        make_identity(nc, identity_h)

        # ---- PE warm-up: dense N=512 matmuls (stream >> LDWEIGHTS so the
        # HAM activity window reads fully-busy and un-throttles ~4us in)
        # while the first input DMAs land.
        warm = consts.tile([128, 512], BF16)
        nc.vector.memset(warm[:], 0.0)

        def emit_warm(n):
            for _ in range(n):
                wps = sc_psum.tile([128, T], FP, tag="sc", name="sc")
                nc.tensor.matmul(
                    wps[:, 0:512], warm[:, 0:128], warm[:], start=True, stop=True
                )

        emit_warm(16)

        # ---- input DMAs (gpsimd SWDGE does fp32->bf16 casts); emission
        # order is the queue order, so earliest-needed data goes first.
        def grouped(src_ap, width, ngrp, col0):
            row_step = src_ap.ap[0][0]
            return bass.AP(
                tensor=src_ap.tensor,
                offset=src_ap.offset + col0,
                ap=[[row_step, 128], [128 * row_step, ngrp], [1, width]],
            )

        xs = [xs_pool.tile([128, 2, C], BF16, tag="xs", name="xs") for _ in range(4)]

        def dma_x(i):
            # one token chunk per DMA: the first transposes start as soon as
            # chunk 0 lands instead of waiting for a 2-chunk transfer
            for k in range(2):
                wdma(
                    xs[i][:, k, :],
                    bass.AP(
                        tensor=xa.tensor,
                        offset=xa.offset + (2 * i + k) * 128 * C,
                        ap=[[C, 128], [1, C]],
                    ),
                )

        wq1_all = wq1_pool.tile([128, CCH, 2 * C], BF16, tag="wq1", name="wq1")

        def dma_wq1(j):
            lo = j * 128
            wdma(wq1_all[:, :, lo : lo + 128], grouped(wqa, 128, CCH, lo))

        wq2_all = wq2_pool.tile([128, CCH, C], BF16, tag="wq2", name="wq2")
        wp_all = wp_pool.tile([128, CCH, C], BF16, tag="wp", name="wp")
        b_bcast = consts.tile([128, C], FP)

        dma_x(0)
        dma_x(1)
        dma_wq1(0)
        dma_wq1(6)
        dma_x(2)
        dma_x(3)
        # wq2 split: the first half (v-cols for heads 0-5) lands ~7us
        # earlier so U(0)/U(1) aren't gated on the full v-weight transfer
        wdma(wq2_all[:, :, 0:384], grouped(wqa, 384, CCH, 2 * C))
        dma_wq1(1)
        dma_wq1(7)
        wdma(wq2_all[:, :, 384:768], grouped(wqa, 384, CCH, 2 * C + 384))
        dma_wq1(2)
        dma_wq1(8)
        wdma(wp_all[:], grouped(wpa, C, CCH, 0))
        dma_wq1(3)
        dma_wq1(9)
        dma_wq1(4)
        dma_wq1(10)
        dma_wq1(5)
        dma_wq1(11)
        nc.sync.dma_start(b_bcast[:], b_bcast_src)

        xT_all = xT_pool.tile([128, CCH, T], BF16, tag="xT", name="xT")

        def emit_xpose(t):
            # PE identity-transpose [128 tok, 128 ch] -> xT_all[:, c, t-slice]
            # 4 PSUM slots (accA+accB) and alternating evac engines keep the
            # transpose stream dense behind the x DMAs.
            for c in range(CCH):
                k = (t * CCH + c) % 4
                pool, tg = ((accA, "accA"), (accB, "accB"))[k // 2]
                ps = pool.tile([128, 512], FP, tag=tg, name=tg)
                psh = ps[:, 0:256].bitcast(BF16)
                nc.tensor.transpose(
                    psh[:, 0:128],
                    xs[t // 2][:, t % 2, c * 128 : (c + 1) * 128],
                    identity_h[:],
                )
                if (t * CCH + c) % 2 == 0:
                    nc.vector.tensor_copy(
                        xT_all[:, c, t * 128 : (t + 1) * 128], psh[:, 0:128]
                    )
                else:
                    nc.scalar.copy(
                        xT_all[:, c, t * 128 : (t + 1) * 128], psh[:, 0:128]
                    )

        for t in range(TC):
            emit_xpose(t)
            emit_warm(2)
        # bridge the gap between the last x-chunk transposes and f1's
        # weights landing — a HAM idle window here re-throttles the clock
        # right as f1(0) starts.
        emit_warm(6)

        qkT = {}

        def emit_f1(j):
            # qkT[j] = w_qkv[:, j-chunk].T @ x^T
            qkT[j] = qk_pool.tile([128, T], BF16, tag="qk", name="qk")
            for nh in range(2):
                ps = accA.tile([128, 512], FP, tag="accA", name="accA")
                for c in range(CCH):
                    nc.tensor.matmul(
                        ps[:],
                        wq1_all[:, c, j * 128 : (j + 1) * 128],
                        xT_all[:, c, nh * 512 : (nh + 1) * 512],
                        start=(c == 0),
                        stop=(c == CCH - 1),
                    )
                nc.vector.tensor_copy(qkT[j][:, nh * 512 : (nh + 1) * 512], ps[:])

        v65 = [
            v_pool.tile([128, 13 * VS], BF16, tag="v65", name="v65")
            for _ in range(TC)
        ]

        def emit_f2(t, nh):
            # v[t] = x[t-chunk] @ w_qkv[:, v-cols]; ones at col 64 of each
            # 96-col head slot (-> denominator rows); pad group 12 covered.
            # nh=0 covers heads 0-5 (slots 0-5) — all that U(0)/U(1) read.
            vt = v65[t]
            if nh == 0:
                nc.vector.memset(
                    vt[:].rearrange("p (g d) -> p g d", d=VS)[:, :, D:], 1.0
                )
                nc.vector.memset(vt[:, 12 * VS : 12 * VS + D], 1.0)
            ps = accA.tile([128, 512], FP, tag="accA", name="accA")
            for c in range(CCH):
                nc.tensor.matmul(
                    ps[:, 0:384],
                    xT_all[:, c, t * 128 : (t + 1) * 128],
                    wq2_all[:, c, nh * 384 : (nh + 1) * 384],
                    start=(c == 0),
                    stop=(c == CCH - 1),
                )
            nc.vector.tensor_copy(
                vt[:, nh * 6 * VS : (nh + 1) * 6 * VS].rearrange(
                    "p (g d) -> p g d", d=VS
                )[:, :, 0:D],
                ps[:, 0:384].rearrange("p (g d) -> p g d", g=6),
            )

        def emit_scores_exp(p, e_pair):
            # Both heads' scoresT for the same 512-col n-range share one
            # [128,1024] PSUM tile ([head 2p | head 2p+1]); the two K=64
            # row-tiles (row_grp h0/h64) issue adjacently -> concurrent
            # streams -> ~2x scores throughput. One exp ACTIVATE per tile.
            for j in range(TC):  # key-token chunks (m)
                psNH = [
                    sc_psum.tile([128, T], FP, tag="sc", name="sc")
                    for _ in range(2)
                ]
                for nh in range(2):
                    for half in range(2):
                        base = 64 * half
                        nc.tensor.matmul(
                            psNH[nh][:, half * 512 : (half + 1) * 512],
                            qkT[6 + p][base : base + 64, j * 128 : (j + 1) * 128],
                            qkT[p][base : base + 64, nh * 512 : (nh + 1) * 512],
                            start=True,
                            stop=True,
                        )
                for nh in range(2):
                    nc.scalar.activation(
                        e_pair[:, j, :, nh * 512 : (nh + 1) * 512],
                        psNH[nh][:],
                        Exp,
                        scale=SCALE,
                    )

        aoT_all = aoT_pool.tile([128, CCH, T], BF16, tag="aoT", name="aoT")
        ao_tiles = {}
        last_atmps = []

        def emit_u(p, e_pair, last=False):
            ao_tiles[p] = ao_pool.tile([128, TC, 128], BF16, tag="ao", name="ao")
            # U^T[d, n] = sum_m v_aug[m, d] expT[m, n]; v stationary
            # (128-wide slice for FWL), expT moving at N=512.
            for half in range(2):
                h = 2 * p + half
                # for the final pair, half 1's normalize chain is routed via
                # ScalarE (exp stream is done) so both halves run in parallel
                alt = last and half == 1
                ups = [
                    accB.tile([128, 512], FP, tag="accB", name="accB")
                    for _ in range(2)
                ]
                for j in range(TC):
                    for nh in range(2):
                        nc.tensor.matmul(
                            ups[nh][:],
                            v65[j][:, h * VS : h * VS + 128],
                            e_pair[:, j, half, nh * 512 : (nh + 1) * 512],
                            start=(j == 0),
                            stop=(j == TC - 1),
                        )
                uT_sb = uT_pool.tile([80, T], BF16, tag="uT", name="uT")
                for nh in range(2):
                    if alt:
                        nc.scalar.copy(
                            uT_sb[:, nh * 512 : (nh + 1) * 512], ups[nh][0:80, :]
                        )
                    else:
                        nc.vector.tensor_copy(
                            uT_sb[:, nh * 512 : (nh + 1) * 512], ups[nh][0:80, :]
                        )
                # token-major via DMA-XBAR: atmp[:, g, k] = uT_sb[k, g*128+p]
                atmp = at_pool.tile([128, TC, 80], BF16, tag="atmp", name="atmp")
                r = r_pool.tile([128, TC], FP, tag="r", name="r")
                if not last:
                    nc.sync.dma_start_transpose(atmp[:], uT_sb[:])
                    ranges = [(0, TC)]
                else:
                    # last pair: two token-range chains fanned over the
                    # sync+scalar DMA queues so aotp/proj stage B can
                    # start on the first half earlier.
                    hr = TC // 2
                    for rng in range(2):
                        eng = nc.sync if (half + rng) % 2 == 0 else nc.scalar
                        eng.dma_start_transpose(
                            atmp[:, rng * hr : (rng + 1) * hr, :],
                            uT_sb[:, rng * 512 : (rng + 1) * 512],
                        )
                    ranges = [(0, hr), (hr, TC)]
                if last:
                    last_atmps.append(atmp)
                for g0, g1 in ranges:
                    nc.vector.reciprocal(r[:, g0:g1], atmp[:, g0:g1, D])
                    rap = r[:, g0:g1]
                    rb = bass.AP(
                        tensor=rap.tensor,
                        offset=rap.offset,
                        ap=[rap.ap[0], rap.ap[1], [0, D]],
                    )
                    nc.vector.tensor_tensor(
                        ao_tiles[p][:, g0:g1, half * D : (half + 1) * D],
                        atmp[:, g0:g1, 0:D],
                        rb,
                        op=Mult,
                    )

        def emit_aotp(c, pool=None, tg="accA"):
            # pair c's ao tile holds proj lhsT chunk c (token-major)
            pool = pool or accA
            for t in range(TC):
                ps = pool.tile([128, 512], FP, tag=tg, name=tg)
                psh = ps[:, 0:256].bitcast(BF16)
                nc.tensor.transpose(
                    psh[:, 0:128],
                    ao_tiles[c][:, t, :],
                    identity_h[:],
                )
                nc.vector.tensor_copy(
                    aoT_all[:, c, t * 128 : (t + 1) * 128], psh[:, 0:128]
                )

        # proj stage A: after pairs 0..3 are done, accumulate their four
        # aoT chunks into bf16 partials yA (with bias); runs mid-attention
        # so only stage B (c=4,5) trails the last pair.
        yA = [
            ya_pool.tile([128, C], BF16, tag="ya", name="ya") for _ in range(TC)
        ]

        def emit_projA():
            for t in range(TC):
                for nh in range(2):
                    k3 = (2 * t + nh) % 3
                    pool = (accA, accB, sc_psum)[k3]
                    tg = ("accA", "accB", "sc")[k3]
                    ps = pool.tile([128, 512], FP, tag=tg, name=tg)
                    for c in range(CA):
                        nc.tensor.matmul(
                            ps[:, 0:384],
                            aoT_all[:, c, t * 128 : (t + 1) * 128],
                            wp_all[:, c, nh * 384 : (nh + 1) * 384],
                            start=(c == 0),
                            stop=(c == CA - 1),
                        )
                    nc.vector.tensor_add(
                        yA[t][:, nh * 384 : (nh + 1) * 384],
                        ps[:, 0:384],
                        b_bcast[:, nh * 384 : (nh + 1) * 384],
                    )

        def emit_projB():
            # stage-A partial (incl. bias) is added on the PE itself via an
            # identity matmul into the accumulation, and the finished rows
            # are DMAed to DRAM straight out of PSUM — no DVE, no staging.
            for t in range(TC):
                for nh in range(2):
                    k3 = (2 * t + nh) % 3
                    pool = (accA, accB, sc_psum)[k3]
                    tg = ("accA", "accB", "sc")[k3]
                    ps = pool.tile([128, 512], FP, tag=tg, name=tg)
                    for c in range(CA, CCH):
                        nc.tensor.matmul(
                            ps[:, 0:384],
                            aoT_all[:, c, t * 128 : (t + 1) * 128],
                            wp_all[:, c, nh * 384 : (nh + 1) * 384],
                            start=(c == CA),
                            stop=False,
                        )
                    nc.tensor.matmul(
                        ps[:, 0:384],
                        identity_h[:],
                        yA[t][:, nh * 384 : (nh + 1) * 384],
                        start=False,
                        stop=True,
                    )
                    y = y_pool.tile([128, 384], FP, tag="y", name="y")
                    if (2 * t + nh) % 2 == 0:
                        nc.vector.tensor_copy(y[:], ps[:, 0:384])
                    else:
                        nc.scalar.copy(y[:], ps[:, 0:384])
                    # three DMA queues round-robin so descriptor issue
                    # (~600ns each) never paces the drain
                    eng = (nc.scalar, nc.sync, nc.gpsimd)[(2 * t + nh) % 3]
                    eng.dma_start(
                        bass.AP(
                            tensor=outa.tensor,
                            offset=outa.offset + t * 128 * C + nh * 384,
                            ap=[[C, 128], [1, 384]],
                        ),
                        y[:],
                    )

        # ---- woven emission schedule ----
        emit_f1(0)
        emit_f1(6)
        e_pairs = []

        def new_pair():
            e = exp_pool.tile([128, TC, 2, T], BF16, tag="expT", name="expT")
            e_pairs.append(e)
            return e

        emit_scores_exp(0, new_pair())
        emit_f1(1)
        emit_f1(7)
        emit_scores_exp(1, new_pair())
        # f2 after scores(1) so the pair-1 score matmuls outrank it and the
        # exp stream never starves behind the v-projection burst; nh=0
        # (heads 0-5) first so U(0)/U(1) unblock as early as possible.
        for t in range(TC):
            emit_f2(t, 0)
        for t in range(TC):
            emit_f2(t, 1)
        # u(p-2) leads each iteration: it is fully ready (exp(p-2) done), so
        # it never stalls the FIFO engine queue, and finishing it early
        # frees the e_pair slot that gates exp(p) — keeping the ScalarE exp
        # stream gapless. f1/scores of pair p trail as ready-paced work.
        # aotp is emitted one iteration later than its pair (aotp(p-3) in
        # iteration p): its normalize chain finished a full window ago, so
        # it never stalls the FIFO PE queue, and aotp(3) lands early enough
        # that proj stage A is ready to fill the exp(5) window.
        for p in range(2, NPAIR):
            emit_u(p - 2, e_pairs[p - 2])
            if p >= 3:
                emit_aotp(p - 3)
            emit_f1(p)
            emit_f1(6 + p)
            emit_scores_exp(p, new_pair())
        # exp(4) is done by the time the loop's PE work drains, so u(4) is
        # safe to emit now; proj stage A fills the exp(5) window alongside.
        emit_u(NPAIR - 2, e_pairs[NPAIR - 2])
        emit_aotp(NPAIR - 3)
        emit_projA()
        emit_u(NPAIR - 1, e_pairs[NPAIR - 1], last=True)
        emit_aotp(NPAIR - 2)
        # keep-warm matmuls gated on the last pair's XBAR transposes: they
        # become ready exactly inside the normalize-chain window, so HAM
        # doesn't re-throttle before aotp(5)/proj stage B.
        for at in last_atmps:
            for _ in range(3):
                wps = sc_psum.tile([128, T], FP, tag="sc", name="sc")
                nc.tensor.matmul(
                    wps[:, 0:480],
                    warm[:, 0:128],
                    at[:, 0:6, :],
                    start=True,
                    stop=True,
                )
        emit_aotp(NPAIR - 1, pool=accB, tg="accB")
        emit_projB()

    nc.finalize()
    return nc

_NC_CACHE = {}

def _get_nc():
    if "nc" not in _NC_CACHE:
        _NC_CACHE["nc"] = build()
    return _NC_CACHE["nc"]

def kernel(x, w_qkv, w_proj, b_proj):
    """Full inputs in, full output out. Shards batch across 8 NeuronCores."""
    assert x.shape == (N_CORES, T, C), x.shape
    nc = _get_nc()
    in_maps = [
        {
            "x": np.ascontiguousarray(x[i], dtype=np.float32),
            "w_qkv": np.ascontiguousarray(w_qkv, dtype=np.float32),
            "w_proj": np.ascontiguousarray(w_proj, dtype=np.float32),
            "b_proj": np.ascontiguousarray(b_proj, dtype=np.float32),
        }
        for i in range(N_CORES)
    ]
    res = run_bass_kernel_spmd(nc, in_maps, list(range(N_CORES)))
    return np.stack([res.results[i]["out"] for i in range(N_CORES)], axis=0)
